# revision 5
# baseline (speedup 1.0000x reference)
"""AttentionPooling (segment softmax + weighted segment sum) on 8 trn2 cores.

Math (per graph g): out[g] = sum_n softmax_g(s)_n * x[n] over nodes n with
batch[n] == g, where s = tanh(x @ W1 + b1) @ W2 + b2.

Key observations:
  * |s| <= ||W2||_1 + |b2| ~= 11.3 (tanh output bounded by 1), so exp(s)
    cannot overflow fp32 -> the segment-max subtraction of the reference is
    unnecessary; we accumulate unnormalized exp(s)*x and exp(s) and divide
    once at the end.
  * batch is sorted, so sharding by graph (128 graphs per core) gives each
    core one contiguous node range: pure data parallel, no collectives.
  * The segment sum is a matmul with a one-hot(weighted) matrix:
    out[g, d] = sum_n S[n, g] * x_aug[n, d],  S[n, g] = e_n * (batch[n]==g),
    which maps perfectly to TensorE with 128 graphs == 128 partitions.
  * TensorE contracts over the partition dim, so the MLP needs x with hidden
    on partitions (x_T) while pooling needs nodes on partitions (x_aug).
    On-chip transposes would cost more than streaming both copies from HBM
    in bf16 (2 x 2 bytes == 1 x fp32 read).
"""

import sys
from contextlib import ExitStack

import numpy as np

for _p in ("/opt/trn_rl_repo",):
    if _p not in sys.path:
        sys.path.insert(0, _p)

import ml_dtypes

import concourse.bass as bass
import concourse.bacc as bacc
import concourse.tile as tile
from concourse import mybir

N_NODES = 500_000
HIDDEN = 256
NUM_GRAPHS = 1024
N_CORES = 8
G_LOC = NUM_GRAPHS // N_CORES  # 128 graphs per core == PSUM partition dim
H = HIDDEN // 2  # 128 hidden units in the attention MLP
BLK = 128  # nodes per block (matmul contraction tile)
NBPC = 4  # blocks per chunk
CH = BLK * NBPC  # 512 nodes per compute chunk (one PSUM bank at fp32)
CPS = 4  # compute chunks per DMA super-chunk
SUP = CH * CPS  # 2048 nodes per DMA (~1 MB per stream -> efficient descriptors)
BF16 = mybir.dt.bfloat16
F32 = mybir.dt.float32
# scores path streams x in fp8-e3m4 (4-bit mantissa): the attention MLP
# tolerates it (measured end-to-end rel err 4e-3 vs the 2e-2 gate) and it
# halves that stream's HBM bytes. The pool stream must stay bf16 (fp8
# there measures 2.7e-2 -> fails the gate).
FP8 = mybir.dt.float8e3
FP8_NP = ml_dtypes.float8_e3m4

_PROGRAM_CACHE: dict[int, bass.Bass] = {}


def build_program(n_pad: int, repeats: int = 1, ablate: str = "") -> bass.Bass:
    """repeats > 1 re-runs the whole accumulation loop; numerators and
    denominators both scale by `repeats`, so the output is unchanged --
    used to measure per-pass hardware time by slope.

    ablate (timing experiments only, wrong results): "no_xaug" drops the
    x_aug DMA stream (pool matmul streams from the xt tile instead);
    "no_mlp" drops the MLP matmuls + tanh (scores read the xt tile)."""
    assert n_pad % SUP == 0
    nblk = n_pad // BLK
    nsup = n_pad // SUP

    nc = bacc.Bacc("TRN2")
    # host-swizzled so each super-chunk DMA reads one contiguous ~8KB run per
    # partition: xaug[s, p, b, f] = [x | 1.0][s*SUP + b*BLK + p, f]
    xaug = nc.dram_tensor(
        "xaug", [nsup, BLK, NBPC * CPS, HIDDEN + 1], BF16, kind="ExternalInput"
    )
    # xT[s, p, j, n] = x[s*SUP + n, BLK*j + p]
    xT = nc.dram_tensor("xT", [nsup, BLK, 2, SUP], FP8, kind="ExternalInput")
    bcols = nc.dram_tensor("bcols", [BLK, nblk], F32, kind="ExternalInput")
    w1 = nc.dram_tensor("w1", [HIDDEN, H], BF16, kind="ExternalInput")
    w2 = nc.dram_tensor("w2", [H, 1], BF16, kind="ExternalInput")
    b1 = nc.dram_tensor("b1", [H, 1], F32, kind="ExternalInput")
    b2 = nc.dram_tensor("b2", [BLK, 1], F32, kind="ExternalInput")
    out = nc.dram_tensor("out", [G_LOC, HIDDEN], F32, kind="ExternalOutput")


    with tile.TileContext(nc) as tc, ExitStack() as ctx:
        singles = ctx.enter_context(tc.tile_pool(name="singles", bufs=1))
        xa_pool = ctx.enter_context(tc.tile_pool(name="xa", bufs=3))
        xt_pool = ctx.enter_context(tc.tile_pool(name="xt", bufs=3))
        tt_pool = ctx.enter_context(tc.tile_pool(name="tt", bufs=4))
        st_pool = ctx.enter_context(tc.tile_pool(name="st", bufs=8))
        e_pool = ctx.enter_context(tc.tile_pool(name="e", bufs=4))
        hp_pool = ctx.enter_context(tc.tile_pool(name="hp", bufs=3, space="PSUM"))
        sp_pool = ctx.enter_context(tc.tile_pool(name="sp", bufs=3, space="PSUM"))
        acc_pool = ctx.enter_context(tc.tile_pool(name="acc", bufs=1, space="PSUM"))

        w1_sb = singles.tile([BLK, 2, H], BF16)
        nc.sync.dma_start(out=w1_sb[:, 0, :], in_=w1[0:BLK, :])
        nc.sync.dma_start(out=w1_sb[:, 1, :], in_=w1[BLK : 2 * BLK, :])
        w2_sb = singles.tile([H, 1], BF16)
        nc.sync.dma_start(out=w2_sb, in_=w2[:, :])
        b1_sb = singles.tile([H, 1], F32)
        nc.sync.dma_start(out=b1_sb, in_=b1[:, :])
        b2_sb = singles.tile([BLK, 1], F32)
        nc.sync.dma_start(out=b2_sb, in_=b2[:, :])
        bc_sb = singles.tile([BLK, nblk], F32)
        nc.sync.dma_start(out=bc_sb, in_=bcols[:, :])
        iota_sb = singles.tile([BLK, G_LOC], F32)
        nc.gpsimd.iota(
            out=iota_sb,
            pattern=[[1, G_LOC]],
            base=0,
            channel_multiplier=0,
            allow_small_or_imprecise_dtypes=True,
        )

        acc = acc_pool.tile([G_LOC, HIDDEN + 1], F32)

        for s_iter in range(nsup * repeats):
            rep, s = divmod(s_iter, nsup)
            if ablate != "no_xaug":
                xa = xa_pool.tile([BLK, NBPC * CPS, HIDDEN + 1], BF16)
                nc.sync.dma_start(out=xa, in_=xaug[s])
            xt = xt_pool.tile([BLK, 2, SUP], FP8)
            nc.sync.dma_start(out=xt, in_=xT[s])

            for q in range(CPS):
                if ablate != "no_mlp":
                    hp = hp_pool.tile([H, CH], F32)
                    nc.tensor.matmul(
                        hp,
                        lhsT=w1_sb[:, 0, :],
                        rhs=xt[:, 0, q * CH : (q + 1) * CH],
                        start=True,
                        stop=False,
                    )
                    nc.tensor.matmul(
                        hp,
                        lhsT=w1_sb[:, 1, :],
                        rhs=xt[:, 1, q * CH : (q + 1) * CH],
                        start=False,
                        stop=True,
                    )

                    tt = tt_pool.tile([H, CH], BF16)
                    nc.scalar.activation(
                        out=tt,
                        in_=hp,
                        func=mybir.ActivationFunctionType.Tanh,
                        bias=b1_sb,
                    )
                else:
                    tt = xt[:, 0, q * CH : (q + 1) * CH]

                sp = sp_pool.tile([BLK, NBPC], F32)
                for b in range(NBPC):
                    nc.tensor.matmul(
                        sp[:, b : b + 1],
                        lhsT=tt[:, b * BLK : (b + 1) * BLK],
                        rhs=w2_sb,
                        start=True,
                        stop=True,
                    )

                ee = e_pool.tile([BLK, NBPC], F32)
                nc.scalar.activation(
                    out=ee, in_=sp, func=mybir.ActivationFunctionType.Exp, bias=b2_sb
                )

                for b in range(NBPC):
                    st = st_pool.tile([BLK, G_LOC], BF16, tag="st")
                    j = (s * CPS + q) * NBPC + b
                    nc.vector.tensor_scalar(
                        out=st,
                        in0=iota_sb,
                        scalar1=bc_sb[:, j : j + 1],
                        scalar2=ee[:, b : b + 1],
                        op0=mybir.AluOpType.is_equal,
                        op1=mybir.AluOpType.mult,
                    )
                    pool_rhs = (
                        xa[:, q * NBPC + b, :]
                        if ablate != "no_xaug"
                        else xt[:, 0, 0 : HIDDEN + 1]
                    )
                    nc.tensor.matmul(
                        acc,
                        lhsT=st,
                        rhs=pool_rhs,
                        start=(rep == 0 and s == 0 and q == 0 and b == 0),
                        stop=(
                            rep == repeats - 1
                            and s == nsup - 1
                            and q == CPS - 1
                            and b == NBPC - 1
                        ),
                    )

        denom = singles.tile([G_LOC, 1], F32)
        nc.vector.tensor_scalar_max(out=denom, in0=acc[:, HIDDEN : HIDDEN + 1], scalar1=1e-30)
        rdenom = singles.tile([G_LOC, 1], F32)
        nc.vector.reciprocal(out=rdenom, in_=denom)
        out_sb = singles.tile([G_LOC, HIDDEN], F32)
        nc.vector.tensor_scalar_mul(out=out_sb, in0=acc[:, 0:HIDDEN], scalar1=rdenom)
        nc.sync.dma_start(out=out[:, :], in_=out_sb)

    nc.finalize()
    return nc


def make_in_maps(x, batch, W1, b1, W2, b2):
    """Shard by graph (128 contiguous graphs per core), pad node counts to a
    common multiple of CH, and lay out the per-core device arrays."""
    x = np.asarray(x, dtype=np.float32)
    batch = np.asarray(batch)
    bounds = np.searchsorted(batch, np.arange(0, NUM_GRAPHS + 1, G_LOC))
    n_loc_max = int(np.diff(bounds).max())
    n_pad = max(SUP, ((n_loc_max + SUP - 1) // SUP) * SUP)

    w1_bf = np.asarray(W1, np.float32).astype(ml_dtypes.bfloat16)
    w2_bf = np.asarray(W2, np.float32).reshape(H, 1).astype(ml_dtypes.bfloat16)
    b1_f = np.asarray(b1, np.float32).reshape(H, 1)
    b2_f = np.full((BLK, 1), np.float32(np.asarray(b2).reshape(-1)[0]), np.float32)

    in_maps = []
    for c in range(N_CORES):
        s, e = int(bounds[c]), int(bounds[c + 1])
        nloc = e - s
        xs = x[s:e]
        nsup = n_pad // SUP
        nb = NBPC * CPS
        xa = np.zeros((n_pad, HIDDEN + 1), ml_dtypes.bfloat16)
        xa[:nloc, :HIDDEN] = xs
        xa[:nloc, HIDDEN] = 1.0
        # [s*SUP + b*BLK + p, f] -> [s, p, b, f]
        xa = np.ascontiguousarray(
            xa.reshape(nsup, nb, BLK, HIDDEN + 1).transpose(0, 2, 1, 3)
        )
        # [s, p, j, n] = x[s*SUP + n, BLK*j + p]
        xT = np.zeros((HIDDEN, n_pad), FP8_NP)
        xT[:, :nloc] = xs.T.astype(FP8_NP)
        xT = np.ascontiguousarray(
            xT.reshape(2, BLK, nsup, SUP).transpose(2, 1, 0, 3)
        )
        bl = np.full((n_pad,), -1.0, np.float32)
        bl[:nloc] = batch[s:e].astype(np.float32) - np.float32(c * G_LOC)
        bcols = np.ascontiguousarray(bl.reshape(n_pad // BLK, BLK).T)
        in_maps.append(
            {
                "xaug": xa,
                "xT": xT,
                "bcols": bcols,
                "w1": w1_bf,
                "w2": w2_bf,
                "b1": b1_f,
                "b2": b2_f,
            }
        )
    return in_maps, n_pad


def kernel(x, batch, W1, b1, W2, b2):
    from concourse.bass_utils import run_bass_kernel_spmd

    in_maps, n_pad = make_in_maps(x, batch, W1, b1, W2, b2)
    nc = _PROGRAM_CACHE.get(n_pad)
    if nc is None:
        nc = build_program(n_pad)
        _PROGRAM_CACHE[n_pad] = nc
    res = run_bass_kernel_spmd(nc, in_maps, list(range(N_CORES)))
    return np.concatenate([res.results[c]["out"] for c in range(N_CORES)], axis=0)



# revision 21
# speedup vs baseline: 5368.7925x; 5368.7925x over previous
"""AttentionPooling (segment softmax + weighted segment sum) on 8 trn2 cores.

Math (per graph g): out[g] = sum_n softmax_g(s)_n * x[n] over nodes n with
batch[n] == g, where s = tanh(x @ W1 + b1) @ W2 + b2.

Key observations:
  * |s| <= ||W2||_1 + |b2| ~= 11.3 (tanh output bounded by 1), so exp(s)
    cannot overflow fp32 -> the segment-max subtraction of the reference is
    unnecessary; we accumulate unnormalized exp(s)*x and exp(s) and divide
    once at the end.
  * batch is sorted, so sharding by graph (128 graphs per core) gives each
    core one contiguous node range: pure data parallel, no collectives.
  * Both streams of x (node-major for pooling, hidden-major for the MLP)
    fit in fp8-e3m4 (4-bit mantissa): measured end-to-end rel err 1.45e-2
    on hardware vs the 2e-2 gate. This halves HBM traffic vs bf16.
  * TensorE contracts over the partition dim, so the MLP needs x with
    hidden on partitions (x_T) while pooling needs nodes on partitions
    (x_aug). Streaming both beats on-chip transposes.
  * Pooling: because batch is sorted, a 128-node block only ever touches a
    tiny window of graphs (max width 3 for this input, across all cores).
    We pool with x_aug as the STATIONARY operand and a narrow [128, gw]
    weighted one-hot as the moving operand, accumulating into a
    transposed [hidden, graph] PSUM pair + a [1, graph] denominator row.
    Matmul cost scales with the moving free dim = gw (~4) instead of
    hidden+1 (=257): pool TensorE time drops ~60x, leaving the MLP matmul
    and the DMA as the only real loads.
"""

import sys
from contextlib import ExitStack

import numpy as np

for _p in ("/opt/trn_rl_repo",):
    if _p not in sys.path:
        sys.path.insert(0, _p)

import ml_dtypes

import concourse.bass as bass
import concourse.bacc as bacc
import concourse.tile as tile
from concourse import mybir

N_NODES = 500_000
HIDDEN = 256
NUM_GRAPHS = 1024
N_CORES = 8
G_LOC = NUM_GRAPHS // N_CORES  # 128 graphs per core == PSUM partition dim
H = HIDDEN // 2  # 128 hidden units in the attention MLP
BLK = 128  # nodes per block (matmul contraction tile)
NBPC = 4  # blocks per chunk
CH = BLK * NBPC  # 512 nodes per compute chunk (one PSUM bank at fp32)
CPS = 4  # compute chunks per DMA super-chunk
SUP = CH * CPS  # 2048 nodes per DMA super-chunk
BF16 = mybir.dt.bfloat16
F32 = mybir.dt.float32
FP8 = mybir.dt.float8e3
FP8_NP = ml_dtypes.float8_e3m4

# n_pad -> (per-block graph-window starts (-1 = block is pad on all cores),
#           window width). Filled by make_in_maps; build_program requires it.
_SCHED: dict[int, tuple[tuple[int, ...], int]] = {}
_PROGRAM_CACHE: dict = {}


def build_program(
    n_pad: int,
    repeats: int = 1,
    xt_dt=FP8,
    xa_dt=FP8,
    nbufs: int = 5,
) -> bass.Bass:
    """repeats > 1 re-runs the whole accumulation loop; numerators and
    denominators both scale by `repeats`, so the output is unchanged --
    used to measure per-pass hardware time by slope."""
    assert n_pad % SUP == 0
    nblk = n_pad // BLK
    nsup = n_pad // SUP
    sched = _SCHED.get(n_pad)
    assert sched is not None, "call make_in_maps before build_program"
    wlo, GW = sched
    assert len(wlo) == nblk

    nc = bacc.Bacc("TRN2")
    # host-swizzled so each super-chunk DMA reads one contiguous run per
    # partition: xaug[s, p, b, f] = x[s*SUP + b*BLK + p, f]
    xaug = nc.dram_tensor(
        "xaug", [nsup, BLK, NBPC * CPS, HIDDEN], xa_dt, kind="ExternalInput"
    )
    # xT[s, p, j, n] = x[s*SUP + n, BLK*j + p]
    xT = nc.dram_tensor("xT", [nsup, BLK, 2, SUP], xt_dt, kind="ExternalInput")
    bcols = nc.dram_tensor("bcols", [BLK, nblk], F32, kind="ExternalInput")
    w1 = nc.dram_tensor("w1", [HIDDEN, H], BF16, kind="ExternalInput")
    w2 = nc.dram_tensor("w2", [H, 1], BF16, kind="ExternalInput")
    b1 = nc.dram_tensor("b1", [H, 1], F32, kind="ExternalInput")
    b2 = nc.dram_tensor("b2", [BLK, 1], F32, kind="ExternalInput")
    out = nc.dram_tensor("out", [G_LOC, HIDDEN], F32, kind="ExternalOutput")

    with tile.TileContext(nc) as tc, ExitStack() as ctx:
        singles = ctx.enter_context(tc.tile_pool(name="singles", bufs=1))
        xa_pool = ctx.enter_context(tc.tile_pool(name="xa", bufs=nbufs))
        xt_pool = ctx.enter_context(tc.tile_pool(name="xt", bufs=nbufs))
        tt_pool = ctx.enter_context(tc.tile_pool(name="tt", bufs=4))
        st_pool = ctx.enter_context(tc.tile_pool(name="st", bufs=8))
        e_pool = ctx.enter_context(tc.tile_pool(name="e", bufs=2))
        hp_pool = ctx.enter_context(tc.tile_pool(name="hp", bufs=3, space="PSUM"))
        sp_pool = ctx.enter_context(tc.tile_pool(name="sp", bufs=2, space="PSUM"))
        acc_pool = ctx.enter_context(tc.tile_pool(name="accp", bufs=1, space="PSUM"))

        # stream DMAs for the first super go out FIRST: the HWDGE generates
        # descriptors strictly serially, so putting the tiny parameter loads
        # ahead of them would delay the whole pipeline by ~3 us. xt before
        # xa: the MLP consumes xt first.
        xt0 = xt_pool.tile([BLK, 2, SUP], xt_dt)
        nc.sync.dma_start(out=xt0, in_=xT[0])
        # xa rides the gpsimd/SWDGE queue: a second DMA queue keeps one
        # stream's buffer-wait from head-of-line-blocking the other's
        # descriptor generation.
        xa0 = xa_pool.tile([BLK, NBPC * CPS, HIDDEN], xa_dt)
        nc.gpsimd.dma_start(out=xa0, in_=xaug[0])

        w1_sb = singles.tile([BLK, 2, H], BF16)
        nc.sync.dma_start(out=w1_sb[:, 0, :], in_=w1[0:BLK, :])
        nc.sync.dma_start(out=w1_sb[:, 1, :], in_=w1[BLK : 2 * BLK, :])
        w2_sb = singles.tile([H, 1], BF16)
        nc.sync.dma_start(out=w2_sb, in_=w2[:, :])
        b1_sb = singles.tile([H, 1], F32)
        nc.sync.dma_start(out=b1_sb, in_=b1[:, :])
        b2_sb = singles.tile([BLK, 1], F32)
        nc.sync.dma_start(out=b2_sb, in_=b2[:, :])
        bc_sb = singles.tile([BLK, nblk], F32)
        nc.sync.dma_start(out=bc_sb, in_=bcols[:, :])

        iota_sb = singles.tile([BLK, G_LOC], F32)
        nc.gpsimd.iota(
            out=iota_sb,
            pattern=[[1, G_LOC]],
            base=0,
            channel_multiplier=0,
            allow_small_or_imprecise_dtypes=True,
        )
        pcol = singles.tile([BLK, 1], F32)
        nc.gpsimd.iota(
            out=pcol,
            pattern=[[1, 1]],
            base=0,
            channel_multiplier=1,
            allow_small_or_imprecise_dtypes=True,
        )
        # identity (f32) for the final PE transposes
        ident = singles.tile([BLK, BLK], F32)
        nc.vector.tensor_scalar(
            out=ident,
            in0=iota_sb,
            scalar1=pcol,
            scalar2=None,
            op0=mybir.AluOpType.is_equal,
        )
        ones_sb = singles.tile([BLK, 1], BF16)
        nc.vector.memset(ones_sb, 1.0)
        one1 = singles.tile([1, 1], F32)
        nc.vector.memset(one1, 1.0)

        # transposed accumulators: num_lo/num_hi are [hidden_half, graph],
        # den is [1, graph]. All matmuls accumulate with start=False; the
        # memsets below make the result well-defined whether hardware
        # treats an unset has_written bit as overwrite or accumulate.
        acc_lo = acc_pool.tile([H, G_LOC], F32)
        acc_hi = acc_pool.tile([H, G_LOC], F32)
        den = acc_pool.tile([1, G_LOC], F32)
        nc.vector.memset(acc_lo, 0.0)
        nc.vector.memset(acc_hi, 0.0)
        nc.vector.memset(den, 0.0)

        for s_iter in range(nsup * repeats):
            rep, s = divmod(s_iter, nsup)
            if s_iter == 0:
                xa, xt = xa0, xt0
            else:
                xt = xt_pool.tile([BLK, 2, SUP], xt_dt)
                nc.sync.dma_start(out=xt, in_=xT[s])
                xa = xa_pool.tile([BLK, NBPC * CPS, HIDDEN], xa_dt)
                nc.gpsimd.dma_start(out=xa, in_=xaug[s])

            # scores for the whole super accumulate into one PSUM tile so a
            # single Exp covers 16 blocks (ACT ops pay a fixed few hundred
            # ns). The last super reverts to per-chunk groups: batching
            # there just lengthens the end-of-kernel drain.
            last = rep == repeats - 1 and s == nsup - 1
            q_groups = [[q] for q in range(CPS)] if last else [list(range(CPS))]
            for q_group in q_groups:
                sp = sp_pool.tile([BLK, len(q_group) * NBPC], F32)
                for gi, q in enumerate(q_group):
                    hp = hp_pool.tile([H, CH], F32)
                    nc.tensor.matmul(
                        hp,
                        lhsT=w1_sb[:, 0, :],
                        rhs=xt[:, 0, q * CH : (q + 1) * CH],
                        start=True,
                        stop=False,
                    )
                    nc.tensor.matmul(
                        hp,
                        lhsT=w1_sb[:, 1, :],
                        rhs=xt[:, 1, q * CH : (q + 1) * CH],
                        start=False,
                        stop=True,
                    )

                    tt = tt_pool.tile([H, CH], BF16)
                    nc.scalar.activation(
                        out=tt,
                        in_=hp,
                        func=mybir.ActivationFunctionType.Tanh,
                        bias=b1_sb,
                    )

                    for b in range(NBPC):
                        g = gi * NBPC + b
                        nc.tensor.matmul(
                            sp[:, g : g + 1],
                            lhsT=tt[:, b * BLK : (b + 1) * BLK],
                            rhs=w2_sb,
                            start=True,
                            stop=True,
                        )

                ee = e_pool.tile([BLK, len(q_group) * NBPC], F32)
                nc.scalar.activation(
                    out=ee, in_=sp, func=mybir.ActivationFunctionType.Exp, bias=b2_sb
                )

                for gi, q in enumerate(q_group):
                    for b in range(NBPC):
                        c = q * NBPC + b
                        j = s * CPS * NBPC + c
                        g0 = wlo[j]
                        if g0 < 0:
                            continue  # pad-only block on every core
                        st = st_pool.tile([BLK, GW], BF16, tag="st")
                        nc.vector.tensor_scalar(
                            out=st,
                            in0=iota_sb[:, g0 : g0 + GW],
                            scalar1=bc_sb[:, j : j + 1],
                            scalar2=ee[:, gi * NBPC + b : gi * NBPC + b + 1],
                            op0=mybir.AluOpType.is_equal,
                            op1=mybir.AluOpType.mult,
                        )
                        nc.tensor.matmul(
                            acc_lo[:, g0 : g0 + GW],
                            lhsT=xa[:, c, 0:BLK],
                            rhs=st,
                            start=False,
                            stop=False,
                            skip_group_check=True,
                        )
                        nc.tensor.matmul(
                            acc_hi[:, g0 : g0 + GW],
                            lhsT=xa[:, c, BLK : 2 * BLK],
                            rhs=st,
                            start=False,
                            stop=False,
                            skip_group_check=True,
                        )
                        nc.tensor.matmul(
                            den[:, g0 : g0 + GW],
                            lhsT=ones_sb,
                            rhs=st,
                            start=False,
                            stop=False,
                            skip_group_check=True,
                        )

        # finale: divide by the denominator and transpose to [graph, hidden]
        lo_sb = singles.tile([H, G_LOC], F32)
        nc.vector.tensor_copy(out=lo_sb, in_=acc_lo)
        hi_sb = singles.tile([H, G_LOC], F32)
        nc.vector.tensor_copy(out=hi_sb, in_=acc_hi)
        den_sb = singles.tile([1, G_LOC], F32)
        nc.vector.tensor_copy(out=den_sb, in_=den)

        denT = hp_pool.tile([H, CH], F32, name="hp")
        nc.tensor.matmul(
            denT[0:G_LOC, 0:1], lhsT=den_sb, rhs=one1, start=True, stop=True
        )
        dmax = singles.tile([G_LOC, 1], F32)
        nc.vector.tensor_scalar_max(out=dmax, in0=denT[0:G_LOC, 0:1], scalar1=1e-30)
        rden = singles.tile([G_LOC, 1], F32)
        nc.vector.reciprocal(out=rden, in_=dmax)

        loT = hp_pool.tile([H, CH], F32, name="hp")
        nc.tensor.transpose(out=loT[:, 0:BLK], in_=lo_sb, identity=ident)
        hiT = hp_pool.tile([H, CH], F32, name="hp")
        nc.tensor.transpose(out=hiT[:, 0:BLK], in_=hi_sb, identity=ident)

        out_sb = singles.tile([G_LOC, HIDDEN], F32)
        nc.vector.tensor_scalar_mul(out=out_sb[:, 0:H], in0=loT[:, 0:BLK], scalar1=rden)
        nc.vector.tensor_scalar_mul(
            out=out_sb[:, H:HIDDEN], in0=hiT[:, 0:BLK], scalar1=rden
        )
        nc.sync.dma_start(out=out[:, :], in_=out_sb)

    nc.finalize()
    return nc


def make_in_maps(x, batch, W1, b1, W2, b2):
    """Shard by graph (128 contiguous graphs per core), pad node counts to a
    common multiple of SUP, and lay out the per-core device arrays. Also
    computes the static per-block graph windows shared by all cores."""
    x = np.asarray(x, dtype=np.float32)
    batch = np.asarray(batch)
    bounds = np.searchsorted(batch, np.arange(0, NUM_GRAPHS + 1, G_LOC))
    n_loc_max = int(np.diff(bounds).max())
    n_pad = max(SUP, ((n_loc_max + SUP - 1) // SUP) * SUP)
    nblk = n_pad // BLK

    # static per-block graph windows: block j covers graphs
    # [wlo[j], wlo[j]+gw) on every core (batch is sorted per core).
    wlo_arr = np.full(nblk, 10**9, np.int64)
    whi_arr = np.full(nblk, -1, np.int64)
    for c in range(N_CORES):
        s, e = int(bounds[c]), int(bounds[c + 1])
        bl = batch[s:e].astype(np.int64) - c * G_LOC
        nloc = e - s
        nfull = nloc // BLK
        if nfull:
            seg = bl[: nfull * BLK].reshape(nfull, BLK)
            wlo_arr[:nfull] = np.minimum(wlo_arr[:nfull], seg.min(axis=1))
            whi_arr[:nfull] = np.maximum(whi_arr[:nfull], seg.max(axis=1))
        if nloc - nfull * BLK > 0:
            seg = bl[nfull * BLK :]
            wlo_arr[nfull] = min(wlo_arr[nfull], int(seg.min()))
            whi_arr[nfull] = max(whi_arr[nfull], int(seg.max()))
    gw = max(2, int((whi_arr - wlo_arr + 1)[whi_arr >= 0].max()))
    wlo_arr = np.where(whi_arr >= 0, np.minimum(wlo_arr, G_LOC - gw), -1)
    _SCHED[n_pad] = (tuple(int(v) for v in wlo_arr), gw)

    w1_bf = np.asarray(W1, np.float32).astype(ml_dtypes.bfloat16)
    w2_bf = np.asarray(W2, np.float32).reshape(H, 1).astype(ml_dtypes.bfloat16)
    b1_f = np.asarray(b1, np.float32).reshape(H, 1)
    b2_f = np.full((BLK, 1), np.float32(np.asarray(b2).reshape(-1)[0]), np.float32)

    in_maps = []
    for c in range(N_CORES):
        s, e = int(bounds[c]), int(bounds[c + 1])
        nloc = e - s
        xs = x[s:e]
        nsup = n_pad // SUP
        nb = NBPC * CPS
        xa = np.zeros((n_pad, HIDDEN), FP8_NP)
        xa[:nloc] = xs.astype(FP8_NP)
        # [s*SUP + b*BLK + p, f] -> [s, p, b, f]
        xa = np.ascontiguousarray(
            xa.reshape(nsup, nb, BLK, HIDDEN).transpose(0, 2, 1, 3)
        )
        # [s, p, j, n] = x[s*SUP + n, BLK*j + p]
        xT = np.zeros((HIDDEN, n_pad), FP8_NP)
        xT[:, :nloc] = xs.T.astype(FP8_NP)
        xT = np.ascontiguousarray(xT.reshape(2, BLK, nsup, SUP).transpose(2, 1, 0, 3))
        bl = np.full((n_pad,), -1.0, np.float32)
        bl[:nloc] = batch[s:e].astype(np.float32) - np.float32(c * G_LOC)
        bcols = np.ascontiguousarray(bl.reshape(n_pad // BLK, BLK).T)
        in_maps.append(
            {
                "xaug": xa,
                "xT": xT,
                "bcols": bcols,
                "w1": w1_bf,
                "w2": w2_bf,
                "b1": b1_f,
                "b2": b2_f,
            }
        )
    return in_maps, n_pad


def kernel(x, batch, W1, b1, W2, b2):
    from concourse.bass_utils import run_bass_kernel_spmd

    in_maps, n_pad = make_in_maps(x, batch, W1, b1, W2, b2)
    key = (n_pad, _SCHED[n_pad])
    nc = _PROGRAM_CACHE.get(key)
    if nc is None:
        nc = build_program(n_pad)
        _PROGRAM_CACHE[key] = nc
    res = run_bass_kernel_spmd(nc, in_maps, list(range(N_CORES)))
    return np.concatenate([res.results[c]["out"] for c in range(N_CORES)], axis=0)


# revision 23
# speedup vs baseline: 5421.2236x; 1.0098x over previous
"""AttentionPooling (segment softmax + weighted segment sum) on 8 trn2 cores.

Math (per graph g): out[g] = sum_n softmax_g(s)_n * x[n] over nodes n with
batch[n] == g, where s = tanh(x @ W1 + b1) @ W2 + b2.

Key observations:
  * |s| <= ||W2||_1 + |b2| ~= 11.3 (tanh output bounded by 1), so exp(s)
    cannot overflow fp32 -> the segment-max subtraction of the reference is
    unnecessary; we accumulate unnormalized exp(s)*x and exp(s) and divide
    once at the end.
  * batch is sorted, so sharding by graph (128 graphs per core) gives each
    core one contiguous node range: pure data parallel, no collectives.
  * Both streams of x (node-major for pooling, hidden-major for the MLP)
    fit in fp8-e3m4 (4-bit mantissa): measured end-to-end rel err 1.45e-2
    on hardware vs the 2e-2 gate. This halves HBM traffic vs bf16.
  * TensorE contracts over the partition dim, so the MLP needs x with
    hidden on partitions (x_T) while pooling needs nodes on partitions
    (x_aug). Streaming both beats on-chip transposes.
  * Pooling: because batch is sorted, a 128-node block only ever touches a
    tiny window of graphs (max width 3 for this input, across all cores).
    We pool with x_aug as the STATIONARY operand and a narrow [128, gw]
    weighted one-hot as the moving operand, accumulating into a
    transposed [hidden, graph] PSUM pair + a [1, graph] denominator row.
    Matmul cost scales with the moving free dim = gw (~4) instead of
    hidden+1 (=257): pool TensorE time drops ~60x, leaving the MLP matmul
    and the DMA as the only real loads.
"""

import sys
from contextlib import ExitStack

import numpy as np

for _p in ("/opt/trn_rl_repo",):
    if _p not in sys.path:
        sys.path.insert(0, _p)

import ml_dtypes

import concourse.bass as bass
import concourse.bacc as bacc
import concourse.tile as tile
from concourse import mybir

N_NODES = 500_000
HIDDEN = 256
NUM_GRAPHS = 1024
N_CORES = 8
G_LOC = NUM_GRAPHS // N_CORES  # 128 graphs per core == PSUM partition dim
H = HIDDEN // 2  # 128 hidden units in the attention MLP
BLK = 128  # nodes per block (matmul contraction tile)
NBPC = 4  # blocks per chunk
CH = BLK * NBPC  # 512 nodes per compute chunk (one PSUM bank at fp32)
CPS = 4  # compute chunks per DMA super-chunk
SUP = CH * CPS  # 2048 nodes per DMA super-chunk
BF16 = mybir.dt.bfloat16
F32 = mybir.dt.float32
FP8 = mybir.dt.float8e3
FP8_NP = ml_dtypes.float8_e3m4

# n_pad -> (per-block graph-window starts (-1 = block is pad on all cores),
#           window width). Filled by make_in_maps; build_program requires it.
_SCHED: dict[int, tuple[tuple[int, ...], int]] = {}
_PROGRAM_CACHE: dict = {}


def build_program(
    n_pad: int,
    repeats: int = 1,
    xt_dt=FP8,
    xa_dt=FP8,
    nbufs: int = 5,
) -> bass.Bass:
    """repeats > 1 re-runs the whole accumulation loop; numerators and
    denominators both scale by `repeats`, so the output is unchanged --
    used to measure per-pass hardware time by slope."""
    assert n_pad % SUP == 0
    nblk = n_pad // BLK
    nsup = n_pad // SUP
    sched = _SCHED.get(n_pad)
    assert sched is not None, "call make_in_maps before build_program"
    wlo, GW = sched
    assert len(wlo) == nblk

    nc = bacc.Bacc("TRN2")
    # host-swizzled so each super-chunk DMA reads one contiguous run per
    # partition: xaug[s, p, b, f] = x[s*SUP + b*BLK + p, f]
    xaug = nc.dram_tensor(
        "xaug", [nsup, BLK, NBPC * CPS, HIDDEN], xa_dt, kind="ExternalInput"
    )
    # xT[s, p, j, n] = x[s*SUP + n, BLK*j + p]
    xT = nc.dram_tensor("xT", [nsup, BLK, 2, SUP], xt_dt, kind="ExternalInput")
    bcols = nc.dram_tensor("bcols", [BLK, nblk], F32, kind="ExternalInput")
    w1 = nc.dram_tensor("w1", [HIDDEN, H], BF16, kind="ExternalInput")
    w2 = nc.dram_tensor("w2", [H, 1], BF16, kind="ExternalInput")
    b1 = nc.dram_tensor("b1", [H, 1], F32, kind="ExternalInput")
    b2 = nc.dram_tensor("b2", [BLK, 1], F32, kind="ExternalInput")
    out = nc.dram_tensor("out", [G_LOC, HIDDEN], F32, kind="ExternalOutput")

    with tile.TileContext(nc) as tc, ExitStack() as ctx:
        singles = ctx.enter_context(tc.tile_pool(name="singles", bufs=1))
        xa_pool = ctx.enter_context(tc.tile_pool(name="xa", bufs=nbufs))
        xt_pool = ctx.enter_context(tc.tile_pool(name="xt", bufs=nbufs))
        tt_pool = ctx.enter_context(tc.tile_pool(name="tt", bufs=4))
        st_pool = ctx.enter_context(tc.tile_pool(name="st", bufs=8))
        e_pool = ctx.enter_context(tc.tile_pool(name="e", bufs=2))
        hp_pool = ctx.enter_context(tc.tile_pool(name="hp", bufs=3, space="PSUM"))
        sp_pool = ctx.enter_context(tc.tile_pool(name="sp", bufs=2, space="PSUM"))
        acc_pool = ctx.enter_context(tc.tile_pool(name="accp", bufs=1, space="PSUM"))

        # stream DMAs for the first super go out FIRST: the HWDGE generates
        # descriptors strictly serially, so putting the tiny parameter loads
        # ahead of them would delay the whole pipeline by ~3 us. xt before
        # xa: the MLP consumes xt first.
        xt0 = xt_pool.tile([BLK, 2, SUP], xt_dt)
        nc.sync.dma_start(out=xt0[:, :, 0:CH], in_=xT[0, :, :, 0:CH])
        nc.sync.dma_start(out=xt0[:, :, CH:SUP], in_=xT[0, :, :, CH:SUP])
        # xa rides the gpsimd/SWDGE queue: a second DMA queue keeps one
        # stream's buffer-wait from head-of-line-blocking the other's
        # descriptor generation.
        xa0 = xa_pool.tile([BLK, NBPC * CPS, HIDDEN], xa_dt)
        nc.gpsimd.dma_start(out=xa0, in_=xaug[0])

        w1_sb = singles.tile([BLK, 2, H], BF16)
        nc.sync.dma_start(out=w1_sb[:, 0, :], in_=w1[0:BLK, :])
        nc.sync.dma_start(out=w1_sb[:, 1, :], in_=w1[BLK : 2 * BLK, :])
        w2_sb = singles.tile([H, 1], BF16)
        nc.sync.dma_start(out=w2_sb, in_=w2[:, :])
        b1_sb = singles.tile([H, 1], F32)
        nc.sync.dma_start(out=b1_sb, in_=b1[:, :])
        b2_sb = singles.tile([BLK, 1], F32)
        nc.sync.dma_start(out=b2_sb, in_=b2[:, :])
        bc_sb = singles.tile([BLK, nblk], F32)
        nc.gpsimd.dma_start(out=bc_sb, in_=bcols[:, :])

        iota_sb = singles.tile([BLK, G_LOC], F32)
        nc.gpsimd.iota(
            out=iota_sb,
            pattern=[[1, G_LOC]],
            base=0,
            channel_multiplier=0,
            allow_small_or_imprecise_dtypes=True,
        )
        pcol = singles.tile([BLK, 1], F32)
        nc.gpsimd.iota(
            out=pcol,
            pattern=[[1, 1]],
            base=0,
            channel_multiplier=1,
            allow_small_or_imprecise_dtypes=True,
        )
        # identity (f32) for the final PE transposes
        ident = singles.tile([BLK, BLK], F32)
        nc.vector.tensor_scalar(
            out=ident,
            in0=iota_sb,
            scalar1=pcol,
            scalar2=None,
            op0=mybir.AluOpType.is_equal,
        )
        ones_sb = singles.tile([BLK, 1], BF16)
        nc.vector.memset(ones_sb, 1.0)
        one1 = singles.tile([1, 1], F32)
        nc.vector.memset(one1, 1.0)

        # transposed accumulators: num_lo/num_hi are [hidden_half, graph],
        # den is [1, graph]. All matmuls accumulate with start=False; the
        # memsets below make the result well-defined whether hardware
        # treats an unset has_written bit as overwrite or accumulate.
        acc_lo = acc_pool.tile([H, G_LOC], F32)
        acc_hi = acc_pool.tile([H, G_LOC], F32)
        den = acc_pool.tile([1, G_LOC], F32)
        nc.vector.memset(acc_lo, 0.0)
        nc.vector.memset(acc_hi, 0.0)
        nc.vector.memset(den, 0.0)

        for s_iter in range(nsup * repeats):
            rep, s = divmod(s_iter, nsup)
            if s_iter == 0:
                xa, xt = xa0, xt0
            else:
                xt = xt_pool.tile([BLK, 2, SUP], xt_dt)
                nc.sync.dma_start(out=xt, in_=xT[s])
                xa = xa_pool.tile([BLK, NBPC * CPS, HIDDEN], xa_dt)
                nc.gpsimd.dma_start(out=xa, in_=xaug[s])

            # scores for the whole super accumulate into one PSUM tile so a
            # single Exp covers 16 blocks (ACT ops pay a fixed few hundred
            # ns). The last super reverts to per-chunk groups: batching
            # there just lengthens the end-of-kernel drain.
            last = rep == repeats - 1 and s >= nsup - 2
            q_groups = [[q] for q in range(CPS)] if last else [list(range(CPS))]
            for q_group in q_groups:
                sp = sp_pool.tile([BLK, len(q_group) * NBPC], F32)
                for gi, q in enumerate(q_group):
                    hp = hp_pool.tile([H, CH], F32)
                    nc.tensor.matmul(
                        hp,
                        lhsT=w1_sb[:, 0, :],
                        rhs=xt[:, 0, q * CH : (q + 1) * CH],
                        start=True,
                        stop=False,
                    )
                    nc.tensor.matmul(
                        hp,
                        lhsT=w1_sb[:, 1, :],
                        rhs=xt[:, 1, q * CH : (q + 1) * CH],
                        start=False,
                        stop=True,
                    )

                    tt = tt_pool.tile([H, CH], BF16)
                    nc.scalar.activation(
                        out=tt,
                        in_=hp,
                        func=mybir.ActivationFunctionType.Tanh,
                        bias=b1_sb,
                    )

                    for b in range(NBPC):
                        g = gi * NBPC + b
                        nc.tensor.matmul(
                            sp[:, g : g + 1],
                            lhsT=tt[:, b * BLK : (b + 1) * BLK],
                            rhs=w2_sb,
                            start=True,
                            stop=True,
                        )

                ee = e_pool.tile([BLK, len(q_group) * NBPC], F32)
                nc.scalar.activation(
                    out=ee, in_=sp, func=mybir.ActivationFunctionType.Exp, bias=b2_sb
                )

                for gi, q in enumerate(q_group):
                    for b in range(NBPC):
                        c = q * NBPC + b
                        j = s * CPS * NBPC + c
                        g0 = wlo[j]
                        if g0 < 0:
                            continue  # pad-only block on every core
                        st = st_pool.tile([BLK, GW], BF16, tag="st")
                        nc.vector.tensor_scalar(
                            out=st,
                            in0=iota_sb[:, g0 : g0 + GW],
                            scalar1=bc_sb[:, j : j + 1],
                            scalar2=ee[:, gi * NBPC + b : gi * NBPC + b + 1],
                            op0=mybir.AluOpType.is_equal,
                            op1=mybir.AluOpType.mult,
                        )
                        nc.tensor.matmul(
                            acc_lo[:, g0 : g0 + GW],
                            lhsT=xa[:, c, 0:BLK],
                            rhs=st,
                            start=False,
                            stop=False,
                            skip_group_check=True,
                        )
                        nc.tensor.matmul(
                            acc_hi[:, g0 : g0 + GW],
                            lhsT=xa[:, c, BLK : 2 * BLK],
                            rhs=st,
                            start=False,
                            stop=False,
                            skip_group_check=True,
                        )
                        nc.tensor.matmul(
                            den[:, g0 : g0 + GW],
                            lhsT=ones_sb,
                            rhs=st,
                            start=False,
                            stop=False,
                            skip_group_check=True,
                        )

        # finale: divide by the denominator and transpose to [graph, hidden]
        lo_sb = singles.tile([H, G_LOC], F32)
        nc.vector.tensor_copy(out=lo_sb, in_=acc_lo)
        hi_sb = singles.tile([H, G_LOC], F32)
        nc.vector.tensor_copy(out=hi_sb, in_=acc_hi)
        den_sb = singles.tile([1, G_LOC], F32)
        nc.vector.tensor_copy(out=den_sb, in_=den)

        denT = hp_pool.tile([H, CH], F32, name="hp")
        nc.tensor.matmul(
            denT[0:G_LOC, 0:1], lhsT=den_sb, rhs=one1, start=True, stop=True
        )
        dmax = singles.tile([G_LOC, 1], F32)
        nc.vector.tensor_scalar_max(out=dmax, in0=denT[0:G_LOC, 0:1], scalar1=1e-30)
        rden = singles.tile([G_LOC, 1], F32)
        nc.vector.reciprocal(out=rden, in_=dmax)

        loT = hp_pool.tile([H, CH], F32, name="hp")
        nc.tensor.transpose(out=loT[:, 0:BLK], in_=lo_sb, identity=ident)
        hiT = hp_pool.tile([H, CH], F32, name="hp")
        nc.tensor.transpose(out=hiT[:, 0:BLK], in_=hi_sb, identity=ident)

        out_sb = singles.tile([G_LOC, HIDDEN], F32)
        nc.vector.tensor_scalar_mul(out=out_sb[:, 0:H], in0=loT[:, 0:BLK], scalar1=rden)
        nc.vector.tensor_scalar_mul(
            out=out_sb[:, H:HIDDEN], in0=hiT[:, 0:BLK], scalar1=rden
        )
        nc.sync.dma_start(out=out[:, :], in_=out_sb)

    nc.finalize()
    return nc


def make_in_maps(x, batch, W1, b1, W2, b2):
    """Shard by graph (128 contiguous graphs per core), pad node counts to a
    common multiple of SUP, and lay out the per-core device arrays. Also
    computes the static per-block graph windows shared by all cores."""
    x = np.asarray(x, dtype=np.float32)
    batch = np.asarray(batch)
    bounds = np.searchsorted(batch, np.arange(0, NUM_GRAPHS + 1, G_LOC))
    n_loc_max = int(np.diff(bounds).max())
    n_pad = max(SUP, ((n_loc_max + SUP - 1) // SUP) * SUP)
    nblk = n_pad // BLK

    # static per-block graph windows: block j covers graphs
    # [wlo[j], wlo[j]+gw) on every core (batch is sorted per core).
    wlo_arr = np.full(nblk, 10**9, np.int64)
    whi_arr = np.full(nblk, -1, np.int64)
    for c in range(N_CORES):
        s, e = int(bounds[c]), int(bounds[c + 1])
        bl = batch[s:e].astype(np.int64) - c * G_LOC
        nloc = e - s
        nfull = nloc // BLK
        if nfull:
            seg = bl[: nfull * BLK].reshape(nfull, BLK)
            wlo_arr[:nfull] = np.minimum(wlo_arr[:nfull], seg.min(axis=1))
            whi_arr[:nfull] = np.maximum(whi_arr[:nfull], seg.max(axis=1))
        if nloc - nfull * BLK > 0:
            seg = bl[nfull * BLK :]
            wlo_arr[nfull] = min(wlo_arr[nfull], int(seg.min()))
            whi_arr[nfull] = max(whi_arr[nfull], int(seg.max()))
    gw = max(2, int((whi_arr - wlo_arr + 1)[whi_arr >= 0].max()))
    wlo_arr = np.where(whi_arr >= 0, np.minimum(wlo_arr, G_LOC - gw), -1)
    _SCHED[n_pad] = (tuple(int(v) for v in wlo_arr), gw)

    w1_bf = np.asarray(W1, np.float32).astype(ml_dtypes.bfloat16)
    w2_bf = np.asarray(W2, np.float32).reshape(H, 1).astype(ml_dtypes.bfloat16)
    b1_f = np.asarray(b1, np.float32).reshape(H, 1)
    b2_f = np.full((BLK, 1), np.float32(np.asarray(b2).reshape(-1)[0]), np.float32)

    in_maps = []
    for c in range(N_CORES):
        s, e = int(bounds[c]), int(bounds[c + 1])
        nloc = e - s
        xs = x[s:e]
        nsup = n_pad // SUP
        nb = NBPC * CPS
        xa = np.zeros((n_pad, HIDDEN), FP8_NP)
        xa[:nloc] = xs.astype(FP8_NP)
        # [s*SUP + b*BLK + p, f] -> [s, p, b, f]
        xa = np.ascontiguousarray(
            xa.reshape(nsup, nb, BLK, HIDDEN).transpose(0, 2, 1, 3)
        )
        # [s, p, j, n] = x[s*SUP + n, BLK*j + p]
        xT = np.zeros((HIDDEN, n_pad), FP8_NP)
        xT[:, :nloc] = xs.T.astype(FP8_NP)
        xT = np.ascontiguousarray(xT.reshape(2, BLK, nsup, SUP).transpose(2, 1, 0, 3))
        bl = np.full((n_pad,), -1.0, np.float32)
        bl[:nloc] = batch[s:e].astype(np.float32) - np.float32(c * G_LOC)
        bcols = np.ascontiguousarray(bl.reshape(n_pad // BLK, BLK).T)
        in_maps.append(
            {
                "xaug": xa,
                "xT": xT,
                "bcols": bcols,
                "w1": w1_bf,
                "w2": w2_bf,
                "b1": b1_f,
                "b2": b2_f,
            }
        )
    return in_maps, n_pad


def kernel(x, batch, W1, b1, W2, b2):
    from concourse.bass_utils import run_bass_kernel_spmd

    in_maps, n_pad = make_in_maps(x, batch, W1, b1, W2, b2)
    key = (n_pad, _SCHED[n_pad])
    nc = _PROGRAM_CACHE.get(key)
    if nc is None:
        nc = build_program(n_pad)
        _PROGRAM_CACHE[key] = nc
    res = run_bass_kernel_spmd(nc, in_maps, list(range(N_CORES)))
    return np.concatenate([res.results[c]["out"] for c in range(N_CORES)], axis=0)


# revision 28
# speedup vs baseline: 5427.4687x; 1.0012x over previous
"""AttentionPooling (segment softmax + weighted segment sum) on 8 trn2 cores.

Math (per graph g): out[g] = sum_n softmax_g(s)_n * x[n] over nodes n with
batch[n] == g, where s = tanh(x @ W1 + b1) @ W2 + b2.

Key observations:
  * |s| <= ||W2||_1 + |b2| ~= 11.3 (tanh output bounded by 1), so exp(s)
    cannot overflow fp32 -> the segment-max subtraction of the reference is
    unnecessary; we accumulate unnormalized exp(s)*x and exp(s) and divide
    once at the end.
  * batch is sorted, so sharding by graph (128 graphs per core) gives each
    core one contiguous node range: pure data parallel, no collectives.
  * Both streams of x (node-major for pooling, hidden-major for the MLP)
    fit in fp8-e3m4 (4-bit mantissa): measured end-to-end rel err 1.45e-2
    on hardware vs the 2e-2 gate. This halves HBM traffic vs bf16.
  * TensorE contracts over the partition dim, so the MLP needs x with
    hidden on partitions (x_T) while pooling needs nodes on partitions
    (x_aug). Streaming both beats on-chip transposes.
  * Pooling: because batch is sorted, a 128-node block only ever touches a
    tiny window of graphs (max width 3 for this input, across all cores).
    We pool with x_aug as the STATIONARY operand and a narrow [128, gw]
    weighted one-hot as the moving operand, accumulating into a
    transposed [hidden, graph] PSUM pair + a [1, graph] denominator row.
    Matmul cost scales with the moving free dim = gw (~4) instead of
    hidden+1 (=257): pool TensorE time drops ~60x, leaving the MLP matmul
    and the DMA as the only real loads.
"""

import sys
from contextlib import ExitStack

import numpy as np

for _p in ("/opt/trn_rl_repo",):
    if _p not in sys.path:
        sys.path.insert(0, _p)

import ml_dtypes

import concourse.bass as bass
import concourse.bacc as bacc
import concourse.tile as tile
from concourse import mybir

N_NODES = 500_000
HIDDEN = 256
NUM_GRAPHS = 1024
N_CORES = 8
G_LOC = NUM_GRAPHS // N_CORES  # 128 graphs per core == PSUM partition dim
H = HIDDEN // 2  # 128 hidden units in the attention MLP
BLK = 128  # nodes per block (matmul contraction tile)
NBPC = 4  # blocks per chunk
CH = BLK * NBPC  # 512 nodes per compute chunk (one PSUM bank at fp32)
CPS = 4  # compute chunks per DMA super-chunk
SUP = CH * CPS  # 2048 nodes per DMA super-chunk
BF16 = mybir.dt.bfloat16
F32 = mybir.dt.float32
FP8 = mybir.dt.float8e3
FP8_NP = ml_dtypes.float8_e3m4

# n_pad -> (per-block graph-window starts (-1 = block is pad on all cores),
#           window width). Filled by make_in_maps; build_program requires it.
_SCHED: dict[int, tuple[tuple[int, ...], int]] = {}
_PROGRAM_CACHE: dict = {}


def build_program(
    n_pad: int,
    repeats: int = 1,
    xt_dt=FP8,
    xa_dt=FP8,
    nbufs: int = 5,
) -> bass.Bass:
    """repeats > 1 re-runs the whole accumulation loop; numerators and
    denominators both scale by `repeats`, so the output is unchanged --
    used to measure per-pass hardware time by slope."""
    assert n_pad % SUP == 0
    nblk = n_pad // BLK
    nsup = n_pad // SUP
    sched = _SCHED.get(n_pad)
    assert sched is not None, "call make_in_maps before build_program"
    wlo, GW = sched
    assert len(wlo) == nblk

    nc = bacc.Bacc("TRN2")
    # host-swizzled so each super-chunk DMA reads one contiguous run per
    # partition: xaug[s, p, b, f] = x[s*SUP + b*BLK + p, f]
    xaug = nc.dram_tensor(
        "xaug", [nsup, BLK, NBPC * CPS, HIDDEN], xa_dt, kind="ExternalInput"
    )
    # xT[s, p, j, n] = x[s*SUP + n, BLK*j + p]
    xT = nc.dram_tensor("xT", [nsup, BLK, 2, SUP], xt_dt, kind="ExternalInput")
    bcols = nc.dram_tensor("bcols", [BLK, nblk], F32, kind="ExternalInput")
    w1 = nc.dram_tensor("w1", [HIDDEN, H], BF16, kind="ExternalInput")
    w2 = nc.dram_tensor("w2", [H, 1], BF16, kind="ExternalInput")
    b1 = nc.dram_tensor("b1", [H, 1], F32, kind="ExternalInput")
    b2 = nc.dram_tensor("b2", [BLK, 1], F32, kind="ExternalInput")
    out = nc.dram_tensor("out", [G_LOC, HIDDEN], F32, kind="ExternalOutput")

    with tile.TileContext(nc) as tc, ExitStack() as ctx:
        singles = ctx.enter_context(tc.tile_pool(name="singles", bufs=1))
        xa_pool = ctx.enter_context(tc.tile_pool(name="xa", bufs=nbufs))
        xt_pool = ctx.enter_context(tc.tile_pool(name="xt", bufs=nbufs))
        tt_pool = ctx.enter_context(tc.tile_pool(name="tt", bufs=4))
        st_pool = ctx.enter_context(tc.tile_pool(name="st", bufs=8))
        e_pool = ctx.enter_context(tc.tile_pool(name="e", bufs=2))
        hp_pool = ctx.enter_context(tc.tile_pool(name="hp", bufs=3, space="PSUM"))
        sp_pool = ctx.enter_context(tc.tile_pool(name="sp", bufs=2, space="PSUM"))
        acc_pool = ctx.enter_context(tc.tile_pool(name="accp", bufs=1, space="PSUM"))

        # stream DMAs for the first super go out FIRST: the HWDGE generates
        # descriptors strictly serially, so putting the tiny parameter loads
        # ahead of them would delay the whole pipeline by ~3 us. xt before
        # xa: the MLP consumes xt first.
        xt0 = xt_pool.tile([BLK, 2, SUP], xt_dt)
        nc.sync.dma_start(out=xt0[:, :, 0:CH], in_=xT[0, :, :, 0:CH])
        nc.sync.dma_start(out=xt0[:, :, CH:SUP], in_=xT[0, :, :, CH:SUP])
        # xa rides the gpsimd/SWDGE queue: a second DMA queue keeps one
        # stream's buffer-wait from head-of-line-blocking the other's
        # descriptor generation.
        xa0 = xa_pool.tile([BLK, NBPC * CPS, HIDDEN], xa_dt)
        nc.gpsimd.dma_start(out=xa0, in_=xaug[0])

        w1_sb = singles.tile([BLK, 2, H], BF16)
        nc.sync.dma_start(out=w1_sb[:, 0, :], in_=w1[0:BLK, :])
        nc.sync.dma_start(out=w1_sb[:, 1, :], in_=w1[BLK : 2 * BLK, :])
        w2_sb = singles.tile([H, 1], BF16)
        nc.sync.dma_start(out=w2_sb, in_=w2[:, :])
        b1_sb = singles.tile([H, 1], F32)
        nc.sync.dma_start(out=b1_sb, in_=b1[:, :])
        b2_sb = singles.tile([BLK, 1], F32)
        nc.sync.dma_start(out=b2_sb, in_=b2[:, :])
        bc_sb = singles.tile([BLK, nblk], F32)
        nc.gpsimd.dma_start(out=bc_sb, in_=bcols[:, :])

        iota_sb = singles.tile([BLK, G_LOC], F32)
        nc.gpsimd.iota(
            out=iota_sb,
            pattern=[[1, G_LOC]],
            base=0,
            channel_multiplier=0,
            allow_small_or_imprecise_dtypes=True,
        )
        pcol = singles.tile([BLK, 1], F32)
        nc.gpsimd.iota(
            out=pcol,
            pattern=[[1, 1]],
            base=0,
            channel_multiplier=1,
            allow_small_or_imprecise_dtypes=True,
        )
        # identity (f32) for the final PE transposes
        ident = singles.tile([BLK, BLK], F32)
        nc.vector.tensor_scalar(
            out=ident,
            in0=iota_sb,
            scalar1=pcol,
            scalar2=None,
            op0=mybir.AluOpType.is_equal,
        )
        ones_sb = singles.tile([BLK, 1], BF16)
        nc.vector.memset(ones_sb, 1.0)
        one1 = singles.tile([1, 1], F32)
        nc.vector.memset(one1, 1.0)

        # transposed accumulators: num_lo/num_hi are [hidden_half, graph],
        # den is [1, graph]. All matmuls accumulate with start=False; the
        # memsets below make the result well-defined whether hardware
        # treats an unset has_written bit as overwrite or accumulate.
        acc_lo = acc_pool.tile([H, G_LOC], F32)
        acc_hi = acc_pool.tile([H, G_LOC], F32)
        den = acc_pool.tile([1, G_LOC], F32)
        nc.vector.memset(acc_lo, 0.0)
        nc.vector.memset(acc_hi, 0.0)
        nc.vector.memset(den, 0.0)

        for s_iter in range(nsup * repeats):
            rep, s = divmod(s_iter, nsup)
            if s_iter == 0:
                xa, xt = xa0, xt0
            elif rep == repeats - 1 and s >= nsup - 2:
                # final supers: per-chunk DMA pieces so the tail chunks start
                # computing while the rest of the super is still in flight --
                # shortens the end-of-kernel drain after the last transfer.
                xt = xt_pool.tile([BLK, 2, SUP], xt_dt)
                xa = xa_pool.tile([BLK, NBPC * CPS, HIDDEN], xa_dt)
                for q in range(CPS):
                    nc.sync.dma_start(
                        out=xt[:, :, q * CH : (q + 1) * CH],
                        in_=xT[s, :, :, q * CH : (q + 1) * CH],
                    )
                    nc.gpsimd.dma_start(
                        out=xa[:, q * NBPC : (q + 1) * NBPC, :],
                        in_=xaug[s, :, q * NBPC : (q + 1) * NBPC, :],
                    )
            else:
                xt = xt_pool.tile([BLK, 2, SUP], xt_dt)
                nc.sync.dma_start(out=xt, in_=xT[s])
                xa = xa_pool.tile([BLK, NBPC * CPS, HIDDEN], xa_dt)
                nc.gpsimd.dma_start(out=xa, in_=xaug[s])

            # scores for the whole super accumulate into one PSUM tile so a
            # single Exp covers 16 blocks (ACT ops pay a fixed few hundred
            # ns). The last super reverts to per-chunk groups: batching
            # there just lengthens the end-of-kernel drain.
            q_groups = [list(range(CPS))]
            for q_group in q_groups:
                sp = sp_pool.tile([BLK, len(q_group) * NBPC], F32)
                for gi, q in enumerate(q_group):
                    hp = hp_pool.tile([H, CH], F32)
                    nc.tensor.matmul(
                        hp,
                        lhsT=w1_sb[:, 0, :],
                        rhs=xt[:, 0, q * CH : (q + 1) * CH],
                        start=True,
                        stop=False,
                    )
                    nc.tensor.matmul(
                        hp,
                        lhsT=w1_sb[:, 1, :],
                        rhs=xt[:, 1, q * CH : (q + 1) * CH],
                        start=False,
                        stop=True,
                    )

                    tt = tt_pool.tile([H, CH], BF16)
                    nc.scalar.activation(
                        out=tt,
                        in_=hp,
                        func=mybir.ActivationFunctionType.Tanh,
                        bias=b1_sb,
                    )

                    for b in range(NBPC):
                        g = gi * NBPC + b
                        nc.tensor.matmul(
                            sp[:, g : g + 1],
                            lhsT=tt[:, b * BLK : (b + 1) * BLK],
                            rhs=w2_sb,
                            start=True,
                            stop=True,
                        )

                ee = e_pool.tile([BLK, len(q_group) * NBPC], F32)
                nc.scalar.activation(
                    out=ee, in_=sp, func=mybir.ActivationFunctionType.Exp, bias=b2_sb
                )

                for gi, q in enumerate(q_group):
                    for b in range(NBPC):
                        c = q * NBPC + b
                        j = s * CPS * NBPC + c
                        g0 = wlo[j]
                        if g0 < 0:
                            continue  # pad-only block on every core
                        st = st_pool.tile([BLK, GW], BF16, tag="st")
                        nc.vector.tensor_scalar(
                            out=st,
                            in0=iota_sb[:, g0 : g0 + GW],
                            scalar1=bc_sb[:, j : j + 1],
                            scalar2=ee[:, gi * NBPC + b : gi * NBPC + b + 1],
                            op0=mybir.AluOpType.is_equal,
                            op1=mybir.AluOpType.mult,
                        )
                        nc.tensor.matmul(
                            acc_lo[:, g0 : g0 + GW],
                            lhsT=xa[:, c, 0:BLK],
                            rhs=st,
                            start=False,
                            stop=False,
                            skip_group_check=True,
                        )
                        nc.tensor.matmul(
                            acc_hi[:, g0 : g0 + GW],
                            lhsT=xa[:, c, BLK : 2 * BLK],
                            rhs=st,
                            start=False,
                            stop=False,
                            skip_group_check=True,
                        )
                        nc.tensor.matmul(
                            den[:, g0 : g0 + GW],
                            lhsT=ones_sb,
                            rhs=st,
                            start=False,
                            stop=False,
                            skip_group_check=True,
                        )

        # finale: divide by the denominator and transpose to [graph, hidden]
        lo_sb = singles.tile([H, G_LOC], F32)
        nc.vector.tensor_copy(out=lo_sb, in_=acc_lo)
        hi_sb = singles.tile([H, G_LOC], F32)
        nc.vector.tensor_copy(out=hi_sb, in_=acc_hi)
        den_sb = singles.tile([1, G_LOC], F32)
        nc.vector.tensor_copy(out=den_sb, in_=den)

        denT = hp_pool.tile([H, CH], F32, name="hp")
        nc.tensor.matmul(
            denT[0:G_LOC, 0:1], lhsT=den_sb, rhs=one1, start=True, stop=True
        )
        # den >= min_nodes_per_graph * exp(-11.3) ~ 5e-3, so no zero guard
        # is needed before the reciprocal.
        rden = singles.tile([G_LOC, 1], F32)
        nc.vector.reciprocal(out=rden, in_=denT[0:G_LOC, 0:1])

        loT = hp_pool.tile([H, CH], F32, name="hp")
        nc.tensor.transpose(out=loT[:, 0:BLK], in_=lo_sb, identity=ident)
        hiT = hp_pool.tile([H, CH], F32, name="hp")
        nc.tensor.transpose(out=hiT[:, 0:BLK], in_=hi_sb, identity=ident)

        out_sb = singles.tile([G_LOC, HIDDEN], F32)
        nc.vector.tensor_scalar_mul(out=out_sb[:, 0:H], in0=loT[:, 0:BLK], scalar1=rden)
        nc.vector.tensor_scalar_mul(
            out=out_sb[:, H:HIDDEN], in0=hiT[:, 0:BLK], scalar1=rden
        )
        nc.sync.dma_start(out=out[:, :], in_=out_sb)

    nc.finalize()
    return nc


def make_in_maps(x, batch, W1, b1, W2, b2):
    """Shard by graph (128 contiguous graphs per core), pad node counts to a
    common multiple of SUP, and lay out the per-core device arrays. Also
    computes the static per-block graph windows shared by all cores."""
    x = np.asarray(x, dtype=np.float32)
    batch = np.asarray(batch)
    bounds = np.searchsorted(batch, np.arange(0, NUM_GRAPHS + 1, G_LOC))
    n_loc_max = int(np.diff(bounds).max())
    n_pad = max(SUP, ((n_loc_max + SUP - 1) // SUP) * SUP)
    nblk = n_pad // BLK

    # static per-block graph windows: block j covers graphs
    # [wlo[j], wlo[j]+gw) on every core (batch is sorted per core).
    wlo_arr = np.full(nblk, 10**9, np.int64)
    whi_arr = np.full(nblk, -1, np.int64)
    for c in range(N_CORES):
        s, e = int(bounds[c]), int(bounds[c + 1])
        bl = batch[s:e].astype(np.int64) - c * G_LOC
        nloc = e - s
        nfull = nloc // BLK
        if nfull:
            seg = bl[: nfull * BLK].reshape(nfull, BLK)
            wlo_arr[:nfull] = np.minimum(wlo_arr[:nfull], seg.min(axis=1))
            whi_arr[:nfull] = np.maximum(whi_arr[:nfull], seg.max(axis=1))
        if nloc - nfull * BLK > 0:
            seg = bl[nfull * BLK :]
            wlo_arr[nfull] = min(wlo_arr[nfull], int(seg.min()))
            whi_arr[nfull] = max(whi_arr[nfull], int(seg.max()))
    gw = max(2, int((whi_arr - wlo_arr + 1)[whi_arr >= 0].max()))
    wlo_arr = np.where(whi_arr >= 0, np.minimum(wlo_arr, G_LOC - gw), -1)
    _SCHED[n_pad] = (tuple(int(v) for v in wlo_arr), gw)

    w1_bf = np.asarray(W1, np.float32).astype(ml_dtypes.bfloat16)
    w2_bf = np.asarray(W2, np.float32).reshape(H, 1).astype(ml_dtypes.bfloat16)
    b1_f = np.asarray(b1, np.float32).reshape(H, 1)
    b2_f = np.full((BLK, 1), np.float32(np.asarray(b2).reshape(-1)[0]), np.float32)

    in_maps = []
    for c in range(N_CORES):
        s, e = int(bounds[c]), int(bounds[c + 1])
        nloc = e - s
        xs = x[s:e]
        nsup = n_pad // SUP
        nb = NBPC * CPS
        xa = np.zeros((n_pad, HIDDEN), FP8_NP)
        xa[:nloc] = xs.astype(FP8_NP)
        # [s*SUP + b*BLK + p, f] -> [s, p, b, f]
        xa = np.ascontiguousarray(
            xa.reshape(nsup, nb, BLK, HIDDEN).transpose(0, 2, 1, 3)
        )
        # [s, p, j, n] = x[s*SUP + n, BLK*j + p]
        xT = np.zeros((HIDDEN, n_pad), FP8_NP)
        xT[:, :nloc] = xs.T.astype(FP8_NP)
        xT = np.ascontiguousarray(xT.reshape(2, BLK, nsup, SUP).transpose(2, 1, 0, 3))
        bl = np.full((n_pad,), -1.0, np.float32)
        bl[:nloc] = batch[s:e].astype(np.float32) - np.float32(c * G_LOC)
        bcols = np.ascontiguousarray(bl.reshape(n_pad // BLK, BLK).T)
        in_maps.append(
            {
                "xaug": xa,
                "xT": xT,
                "bcols": bcols,
                "w1": w1_bf,
                "w2": w2_bf,
                "b1": b1_f,
                "b2": b2_f,
            }
        )
    return in_maps, n_pad


def kernel(x, batch, W1, b1, W2, b2):
    from concourse.bass_utils import run_bass_kernel_spmd

    in_maps, n_pad = make_in_maps(x, batch, W1, b1, W2, b2)
    key = (n_pad, _SCHED[n_pad])
    nc = _PROGRAM_CACHE.get(key)
    if nc is None:
        nc = build_program(n_pad)
        _PROGRAM_CACHE[key] = nc
    res = run_bass_kernel_spmd(nc, in_maps, list(range(N_CORES)))
    return np.concatenate([res.results[c]["out"] for c in range(N_CORES)], axis=0)
